# revision 18
# baseline (speedup 1.0000x reference)
"""Self-contained Trainium2 Bass kernel for batched multi-head attention
with interleaved RoPE and a block-causal mask (block size 8).

Shapes (hardcoded): x [8, 1024, 1024] f32, weights [1024, 1024] f32,
freqs_cos/sin [1024, 32] f32 -> out [8, 1024, 1024] f32.

Sharding: data-parallel over batch, one batch element per NeuronCore (8 cores).

Device algorithm (per core, matmuls in bf16):
  - host pre-transposes x -> XT [D, S] and de-interleaves the RoPE pairing by
    permuting wq/wk columns so each head's 64 dims are [32 real | 32 imag].
  - QT = Wq^T XT, KT = Wk^T XT  ([D, S] layouts, head-major rows)
  - RoPE in [d, s] layout: rot = t * cosf + swap32(t) * sinf, with the 32-row
    block swap done by SBUF->SBUF DMA and sign folded into the sinf table.
  - V = XT^T Wv in natural [S, D] layout, stored with a ones-column per head
    (V' [S, 65] per head) so the PV matmul also produces the softmax
    denominator as its row 64.
  - scores transposed: ST[k, q] = (KT_h k-slice)^T @ QT_h per head per
    128-wide k tile, staircase over allowed q only; the two heads of a
    partition tile run concurrently in the PE via row-group packing.
    exp on ACT with the 1/8 scale folded in; block-diagonal mask applied
    multiplicatively.
  - outT[h] = V'^T @ PT accumulated over k tiles in PSUM ([65, S]).
  - normalization per head: rec = reciprocal_approx(den row) (custom DVE op),
    partition-broadcast on GPSIMD, single fused TT multiply PSUM->SBUF.
  - final = outT^T @ Wo streamed back to HBM in f32.

Matmul chains are emitted with independent accumulations interleaved so the
PE's reorder window can hide LDWEIGHTS behind the previous matmul's stream.
"""

import os
import sys
import types

import numpy as np

B, S, D, H, HD, BS = 8, 1024, 1024, 16, 64, 8
P = 128
NT = D // P  # 8 partition tiles
NCORES = 8

LAST_RESULT = None  # BassKernelResults of the most recent run (for test harness)


def _install_axon_hooks():
    """Provide antenv.axon_hooks (NTFF profiling hook) when the image lacks it."""
    if "antenv.axon_hooks" in sys.modules:
        return
    try:
        import antenv
        from trn_agent_boot.trn_boot import _ntff_profile_via_ctypes

        mod = types.ModuleType("antenv.axon_hooks")
        hook = _ntff_profile_via_ctypes("/opt/axon/libaxon_pjrt.so")
        mod.get_axon_ntff_profile_hook = lambda: hook
        mod.set_axon_ntff_profile_hook = lambda h: None
        sys.modules["antenv.axon_hooks"] = mod
        antenv.axon_hooks = mod
    except Exception:
        mod = types.ModuleType("antenv.axon_hooks")
        mod.get_axon_ntff_profile_hook = lambda: None
        mod.set_axon_ntff_profile_hook = lambda h: None
        sys.modules["antenv.axon_hooks"] = mod


_NC_CACHE = {}


def _build_nc():
    """Build and compile the Bass graph (one SPMD program for all 8 cores)."""
    if "nc" in _NC_CACHE:
        return _NC_CACHE["nc"]

    import concourse.mybir as mybir
    import concourse.tile as tile
    from concourse import bacc

    BF = mybir.dt.bfloat16
    F32 = mybir.dt.float32
    MUL = mybir.AluOpType.mult
    ADD = mybir.AluOpType.add
    EXP = mybir.ActivationFunctionType.Exp

    nc = bacc.Bacc("TRN2", target_bir_lowering=False, debug=False)

    xt_d = nc.dram_tensor("xt", [D, S], BF, kind="ExternalInput")
    wq_d = nc.dram_tensor("wq", [D, D], BF, kind="ExternalInput")
    wk_d = nc.dram_tensor("wk", [D, D], BF, kind="ExternalInput")
    wv_d = nc.dram_tensor("wv", [D, D], BF, kind="ExternalInput")
    wo_d = nc.dram_tensor("wo", [D, D], BF, kind="ExternalInput")
    cos_d = nc.dram_tensor("cosf", [P, S], BF, kind="ExternalInput")
    sin_d = nc.dram_tensor("sinf", [P, S], BF, kind="ExternalInput")
    mask_d = nc.dram_tensor("mask", [P, P], BF, kind="ExternalInput")
    out_d = nc.dram_tensor("out", [S, D], F32, kind="ExternalOutput")

    HC = HD + 1  # 65: V columns per head incl. the ones column

    with tile.TileContext(nc) as tc:
        with (
            tc.tile_pool(name="big", bufs=1) as big,
            tc.tile_pool(name="ps", bufs=2, space="PSUM") as ps,
            tc.tile_pool(name="ot_ps", bufs=2, space="PSUM") as ot_ps,
            tc.tile_pool(name="work", bufs=2) as work,
            tc.tile_pool(name="ptp", bufs=2) as ptp,
        ):
            xt = [big.tile([P, S], BF, tag=f"xt{j}", name=f"xt{j}") for j in range(NT)]
            wqt = [big.tile([P, D], BF, tag=f"wq{j}", name=f"wq{j}") for j in range(NT)]
            wkt = [big.tile([P, D], BF, tag=f"wk{j}", name=f"wk{j}") for j in range(NT)]
            wvt = [big.tile([P, D], BF, tag=f"wv{j}", name=f"wv{j}") for j in range(NT)]
            wot = [big.tile([P, D], BF, tag=f"wo{j}", name=f"wo{j}") for j in range(NT)]
            qt = [big.tile([P, S], BF, tag=f"qt{t}", name=f"qt{t}") for t in range(NT)]
            kt = [big.tile([P, S], BF, tag=f"kt{t}", name=f"kt{t}") for t in range(NT)]
            vs = [big.tile([P, H * HC], BF, tag=f"vs{t}", name=f"vs{t}") for t in range(NT)]
            ot = [big.tile([P, S], BF, tag=f"ot{t}", name=f"ot{t}") for t in range(NT)]
            cosf = big.tile([P, S], BF, tag="cosf", name="cosf")
            sinf = big.tile([P, S], BF, tag="sinf", name="sinf")
            maskt = big.tile([P, P], BF, tag="mask", name="mask")

            # load order matters for the compute ramp: xt+wv feed the V
            # projection (needed before any attention), wq/wk next, wo last
            for j in range(NT):
                rs = slice(j * P, (j + 1) * P)
                nc.sync.dma_start(xt[j][:], xt_d[rs, :])
                nc.sync.dma_start(wvt[j][:], wv_d[rs, :])
                nc.sync.dma_start(wqt[j][:], wq_d[rs, :])
                nc.sync.dma_start(wkt[j][:], wk_d[rs, :])
            nc.sync.dma_start(cosf[:], cos_d[:])
            nc.sync.dma_start(sinf[:], sin_d[:])
            nc.sync.dma_start(maskt[:], mask_d[:])
            for j in range(NT):
                rs = slice(j * P, (j + 1) * P)
                nc.sync.dma_start(wot[j][:], wo_d[rs, :])

            for t in range(NT):
                nc.vector.memset(
                    vs[t].rearrange("p (h c) -> p h c", c=HC)[:, :, HD : HD + 1], 1.0
                )

            # ---- V projection first (all of V gates the first head's PV) --
            # one [128,1024] slot per s-tile; its two bank-halves are the two
            # independent 512-wide accumulation chains, interleaved per j.
            for t in range(NT):
                cs = slice(t * P, (t + 1) * P)
                pv = ps.tile([P, 1024], F32, tag="ps", name="pv")
                for j in range(NT):
                    for m in range(2):
                        nc.tensor.matmul(
                            pv[:, m * 512 : (m + 1) * 512],
                            xt[j][:, cs],
                            wvt[j][:, m * 512 : (m + 1) * 512],
                            start=(j == 0), stop=(j == NT - 1),
                        )
                dst = vs[t].rearrange("p (h c) -> p h c", c=HC)[:, :, 0:HD]
                src = pv.rearrange("p (h c) -> p h c", c=HD)
                nc.vector.tensor_copy(dst, src)

            # RoPE helper: per 128-row tile the layout is [h0r, h0i, h1r,
            # h1i] (32 rows each); rot = t*cosf + swap32(t)*sinf (sinf
            # carries the sign)
            def rope(buf_t):
                tr = work.tile([P, S], BF, tag="trot", name="trot")
                for b4 in range(4):
                    sblk = (b4 ^ 1) * 32
                    dblk = b4 * 32
                    nc.sync.dma_start(
                        tr[dblk : dblk + 32, :], buf_t[sblk : sblk + 32, :]
                    )
                nc.vector.tensor_tensor(tr[:], tr[:], sinf[:], op=MUL)
                nc.vector.tensor_tensor(buf_t[:], buf_t[:], cosf[:], op=MUL)
                nc.vector.tensor_tensor(buf_t[:], buf_t[:], tr[:], op=ADD)

            # ---- QT/KT projections per tile; rope immediately per tile ----
            # q and k accumulate in the two bank-halves of one slot; per-j
            # the weights alternate wq/wk so LDWEIGHTS hides behind the other
            # chain's stream.
            for t in range(NT):
                cs = slice(t * P, (t + 1) * P)
                for m in range(2):
                    sl = slice(m * 512, (m + 1) * 512)
                    pqk = ps.tile([P, 1024], F32, tag="ps", name="pqk")
                    for j in range(NT):
                        nc.tensor.matmul(
                            pqk[:, 0:512], wqt[j][:, cs], xt[j][:, sl],
                            start=(j == 0), stop=(j == NT - 1),
                        )
                        nc.tensor.matmul(
                            pqk[:, 512:1024], wkt[j][:, cs], xt[j][:, sl],
                            start=(j == 0), stop=(j == NT - 1),
                        )
                    nc.vector.tensor_copy(qt[t][:, sl], pqk[:, 0:512])
                    nc.vector.tensor_copy(kt[t][:, sl], pqk[:, 512:1024])
                rope(qt[t])
                rope(kt[t])

            # ---- attention per head-pair; scores transposed ST[k, q] ----
            # the two heads (rows 0:64 and 64:128 of tile t) run their ST
            # matmuls concurrently in the PE (row groups 0/1 vs 2/3).
            scale = 1.0 / 8.0
            for t in range(NT):
                # the two heads of tile t occupy disjoint PE row groups
                # (d rows 0:64 and 64:128), so their ST matmuls run
                # concurrently when emitted adjacently.
                pts = {0: [], 1: []}
                for i in range(NT):
                    off = i * P
                    w = S - off
                    pieces = [(off, min(512, w))]
                    if w > 512:
                        pieces.append((off + 512, w - 512))
                    stp = {}
                    for hh in (0, 1):
                        base = hh * HD
                        stp[hh] = ps.tile([P, 1024], F32, tag="ps", name="stp")
                        for (o, wd) in pieces:
                            nc.tensor.matmul(
                                stp[hh][:, o - off : o - off + wd],
                                kt[t][base : base + HD, off : off + P],
                                qt[t][base : base + HD, o : o + wd],
                                start=True, stop=True,
                            )
                    for hh in (0, 1):
                        pt = ptp.tile([P, w], BF, tag=f"pt{i}", name=f"pt{i}")
                        pts[hh].append(pt)
                        nc.scalar.activation(
                            pt[:, :], stp[hh][:, :w], EXP, scale=scale
                        )
                        nc.gpsimd.tensor_tensor(
                            pt[:, 0:P], pt[:, 0:P], maskt[:], op=MUL
                        )
                for hh in (0, 1):
                    h = 2 * t + hh
                    base = hh * HD
                    otp = ot_ps.tile([HC, S], F32, tag="ot", name="otp")
                    for jb in range(2):
                        lo = jb * 512
                        last_i = min(NT - 1, 4 * jb + 3)
                        for i in range(last_i + 1):
                            off = i * P
                            o = max(lo, off)
                            wd = lo + 512 - o
                            nc.tensor.matmul(
                                otp[:, o : o + wd],
                                vs[i][:, h * HC : (h + 1) * HC],
                                pts[hh][i][:, o - off : o - off + wd],
                                start=(i == 0), stop=(i == last_i),
                            )
                    # normalization: rec = 1/den, bcast over 64 partitions,
                    # fused (copy + multiply) PSUM -> SBUF
                    den = work.tile([1, S], F32, tag="den", name="den")
                    nc.vector.tensor_copy(den[:], otp[HD : HD + 1, :])
                    rec = work.tile([1, S], F32, tag="rec", name="rec")
                    nc.vector.reciprocal_approx_fast(rec[:], den[:])
                    bc = work.tile([HD, S], F32, tag="bc", name="bc")
                    nc.gpsimd.partition_broadcast(bc[:], rec[:])
                    nc.vector.tensor_tensor(
                        ot[t][base : base + HD, :], otp[0:HD, :], bc[:], op=MUL
                    )

            # ---- output projection: final[s, :] = sum_i ot[i][:, s]^T wo[i]
            for st in range(NT):
                cs = slice(st * P, (st + 1) * P)
                fp = ps.tile([P, 1024], F32, tag="ps", name="fp")
                for i in range(NT):
                    for m in range(2):
                        sl = slice(m * 512, (m + 1) * 512)
                        nc.tensor.matmul(
                            fp[:, sl], ot[i][:, cs], wot[i][:, sl],
                            start=(i == 0), stop=(i == NT - 1),
                        )
                osb = work.tile([P, 1024], F32, tag="osb", name="osb")
                nc.vector.tensor_copy(osb[:], fp[:])
                nc.sync.dma_start(out_d[cs, :], osb[:])


    nc.compile()
    _NC_CACHE["nc"] = nc
    return nc


def _host_prep(x, wq, wk, wv, wo, freqs_cos, freqs_sin):
    import ml_dtypes

    bf16 = ml_dtypes.bfloat16

    # de-interleave RoPE pairs: permuted col c of head h maps to original
    # column h*64 + (2r if r<32 else 2(r-32)+1)
    r = np.arange(HD)
    src_local = np.where(r < 32, 2 * r, 2 * (r - 32) + 1)
    perm = (np.arange(H)[:, None] * HD + src_local[None, :]).reshape(-1)

    wq_p = np.ascontiguousarray(wq[:, perm]).astype(bf16)
    wk_p = np.ascontiguousarray(wk[:, perm]).astype(bf16)
    wv_c = np.ascontiguousarray(wv).astype(bf16)
    wo_c = np.ascontiguousarray(wo).astype(bf16)

    cos_t = np.ascontiguousarray(freqs_cos.T).astype(np.float32)  # [32, S]
    sin_t = np.ascontiguousarray(freqs_sin.T).astype(np.float32)
    cosf = np.concatenate([cos_t, cos_t, cos_t, cos_t], 0).astype(bf16)  # [128,S]
    sinf = np.concatenate([-sin_t, sin_t, -sin_t, sin_t], 0).astype(bf16)

    kq = np.arange(P)
    mask = ((kq[:, None] // BS) <= (kq[None, :] // BS)).astype(bf16)  # [128,128]

    in_maps = []
    for b in range(NCORES):
        xt = np.ascontiguousarray(x[b].T).astype(bf16)  # [D, S]
        in_maps.append(
            {
                "xt": xt,
                "wq": wq_p,
                "wk": wk_p,
                "wv": wv_c,
                "wo": wo_c,
                "cosf": cosf,
                "sinf": sinf,
                "mask": mask,
            }
        )
    return in_maps


def kernel(x, wq, wk, wv, wo, freqs_cos, freqs_sin):
    global LAST_RESULT
    x = np.asarray(x, dtype=np.float32)
    wq = np.asarray(wq, dtype=np.float32)
    wk = np.asarray(wk, dtype=np.float32)
    wv = np.asarray(wv, dtype=np.float32)
    wo = np.asarray(wo, dtype=np.float32)
    freqs_cos = np.asarray(freqs_cos, dtype=np.float32)
    freqs_sin = np.asarray(freqs_sin, dtype=np.float32)

    trace = bool(os.environ.get("BASS_TRACE"))
    if trace:
        _install_axon_hooks()
        import concourse.bass_utils as bass_utils

        bass_utils.upload_artifacts = lambda tmpdir: tmpdir  # no-egress sandbox

    from concourse.bass_utils import run_bass_kernel_spmd

    nc = _build_nc()
    in_maps = _host_prep(x, wq, wk, wv, wo, freqs_cos, freqs_sin)
    res = run_bass_kernel_spmd(
        nc, in_maps, core_ids=list(range(NCORES)), trace=trace
    )
    LAST_RESULT = res
    out = np.stack([res.results[b]["out"] for b in range(NCORES)], 0)
    return out.astype(np.float32)


# revision 20
# speedup vs baseline: 1.6199x; 1.6199x over previous
"""Self-contained Trainium2 Bass kernel for batched multi-head attention
with interleaved RoPE and a block-causal mask (block size 8).

Shapes (hardcoded): x [8, 1024, 1024] f32, weights [1024, 1024] f32,
freqs_cos/sin [1024, 32] f32 -> out [8, 1024, 1024] f32.

Sharding: data-parallel over batch, one batch element per NeuronCore (8 cores).

Device algorithm (per core, matmuls in bf16):
  - host pre-transposes x -> XT [D, S] and de-interleaves the RoPE pairing by
    permuting wq/wk columns so each head's 64 dims are [32 real | 32 imag].
  - QT = Wq^T XT, KT = Wk^T XT  ([D, S] layouts, head-major rows)
  - RoPE in [d, s] layout: rot = t * cosf + swap32(t) * sinf, with the 32-row
    block swap done by SBUF->SBUF DMA and sign folded into the sinf table.
  - V = XT^T Wv in natural [S, D] layout, stored with a ones-column per head
    (V' [S, 65] per head) so the PV matmul also produces the softmax
    denominator as its row 64.
  - scores transposed: ST[k, q] = (KT_h k-slice)^T @ QT_h per head per
    128-wide k tile, staircase over allowed q only; the two heads of a
    partition tile run concurrently in the PE via row-group packing.
    exp on ACT with the 1/8 scale folded in; block-diagonal mask applied
    multiplicatively.
  - outT[h] = V'^T @ PT accumulated over k tiles in PSUM ([65, S]).
  - normalization per head: rec = reciprocal_approx(den row) (custom DVE op),
    partition-broadcast on GPSIMD, single fused TT multiply PSUM->SBUF.
  - final = outT^T @ Wo streamed back to HBM in f32.

Matmul chains are emitted with independent accumulations interleaved so the
PE's reorder window can hide LDWEIGHTS behind the previous matmul's stream.
"""

import os
import sys
import types

import numpy as np

B, S, D, H, HD, BS = 8, 1024, 1024, 16, 64, 8
P = 128
NT = D // P  # 8 partition tiles
NCORES = 8

LAST_RESULT = None  # BassKernelResults of the most recent run (for test harness)


def _install_axon_hooks():
    """Provide antenv.axon_hooks (NTFF profiling hook) when the image lacks it."""
    if "antenv.axon_hooks" in sys.modules:
        return
    try:
        import antenv
        from trn_agent_boot.trn_boot import _ntff_profile_via_ctypes

        mod = types.ModuleType("antenv.axon_hooks")
        hook = _ntff_profile_via_ctypes("/opt/axon/libaxon_pjrt.so")
        mod.get_axon_ntff_profile_hook = lambda: hook
        mod.set_axon_ntff_profile_hook = lambda h: None
        sys.modules["antenv.axon_hooks"] = mod
        antenv.axon_hooks = mod
    except Exception:
        mod = types.ModuleType("antenv.axon_hooks")
        mod.get_axon_ntff_profile_hook = lambda: None
        mod.set_axon_ntff_profile_hook = lambda h: None
        sys.modules["antenv.axon_hooks"] = mod


_NC_CACHE = {}


def _build_nc():
    """Build and compile the Bass graph (one SPMD program for all 8 cores)."""
    if "nc" in _NC_CACHE:
        return _NC_CACHE["nc"]

    import concourse.mybir as mybir
    import concourse.tile as tile
    from concourse import bacc

    BF = mybir.dt.bfloat16
    F32 = mybir.dt.float32
    MUL = mybir.AluOpType.mult
    ADD = mybir.AluOpType.add
    EXP = mybir.ActivationFunctionType.Exp

    nc = bacc.Bacc("TRN2", target_bir_lowering=False, debug=False)

    xt_d = nc.dram_tensor("xt", [D, S], BF, kind="ExternalInput")
    wq_d = nc.dram_tensor("wq", [D, D], BF, kind="ExternalInput")
    wk_d = nc.dram_tensor("wk", [D, D], BF, kind="ExternalInput")
    wv_d = nc.dram_tensor("wv", [D, D], BF, kind="ExternalInput")
    wo_d = nc.dram_tensor("wo", [D, D], BF, kind="ExternalInput")
    cos_d = nc.dram_tensor("cosf", [P, S], BF, kind="ExternalInput")
    sin_d = nc.dram_tensor("sinf", [P, S], BF, kind="ExternalInput")
    mask_d = nc.dram_tensor("mask", [P, P], BF, kind="ExternalInput")
    out_d = nc.dram_tensor("out", [S, D], F32, kind="ExternalOutput")

    HC = HD + 1  # 65: V columns per head incl. the ones column

    with tile.TileContext(nc) as tc:
        with (
            tc.tile_pool(name="big", bufs=1) as big,
            tc.tile_pool(name="ps", bufs=2, space="PSUM") as ps,
            tc.tile_pool(name="ot_ps", bufs=2, space="PSUM") as ot_ps,
            tc.tile_pool(name="work", bufs=2) as work,
            tc.tile_pool(name="ptp", bufs=2) as ptp,
        ):
            xt = [big.tile([P, S], BF, tag=f"xt{j}", name=f"xt{j}") for j in range(NT)]
            wqt = [big.tile([P, D], BF, tag=f"wq{j}", name=f"wq{j}") for j in range(NT)]
            wkt = [big.tile([P, D], BF, tag=f"wk{j}", name=f"wk{j}") for j in range(NT)]
            wvt = [big.tile([P, D], BF, tag=f"wv{j}", name=f"wv{j}") for j in range(NT)]
            wot = [big.tile([P, D], BF, tag=f"wo{j}", name=f"wo{j}") for j in range(NT)]
            qt = [big.tile([P, S], BF, tag=f"qt{t}", name=f"qt{t}") for t in range(NT)]
            kt = [big.tile([P, S], BF, tag=f"kt{t}", name=f"kt{t}") for t in range(NT)]
            vs = [big.tile([P, H * HC], BF, tag=f"vs{t}", name=f"vs{t}") for t in range(NT)]
            ot = [big.tile([P, S], BF, tag=f"ot{t}", name=f"ot{t}") for t in range(NT)]
            cosf = big.tile([P, S], BF, tag="cosf", name="cosf")
            sinf = big.tile([P, S], BF, tag="sinf", name="sinf")
            maskt = big.tile([P, P], BF, tag="mask", name="mask")

            # load order matters for the compute ramp: xt+wv feed the V
            # projection (needed before any attention), wq/wk next, wo last
            for j in range(NT):
                rs = slice(j * P, (j + 1) * P)
                nc.sync.dma_start(xt[j][:], xt_d[rs, :])
                nc.sync.dma_start(wvt[j][:], wv_d[rs, :])
                nc.sync.dma_start(wqt[j][:], wq_d[rs, :])
                nc.sync.dma_start(wkt[j][:], wk_d[rs, :])
            nc.sync.dma_start(cosf[:], cos_d[:])
            nc.sync.dma_start(sinf[:], sin_d[:])
            nc.sync.dma_start(maskt[:], mask_d[:])
            for j in range(NT):
                rs = slice(j * P, (j + 1) * P)
                nc.sync.dma_start(wot[j][:], wo_d[rs, :])

            for t in range(NT):
                nc.vector.memset(
                    vs[t].rearrange("p (h c) -> p h c", c=HC)[:, :, HD : HD + 1], 1.0
                )

            # ---- V projection first (all of V gates the first head's PV) --
            for t in range(NT):
                cs = slice(t * P, (t + 1) * P)
                for m in range(2):
                    sl = slice(m * 512, (m + 1) * 512)
                    pv = ps.tile([P, 512], F32, tag="ps", name="pv")
                    for j in range(NT):
                        nc.tensor.matmul(
                            pv[:], xt[j][:, cs], wvt[j][:, sl],
                            start=(j == 0), stop=(j == NT - 1),
                        )
                    dst = vs[t].rearrange("p (h c) -> p h c", c=HC)[
                        :, m * 8 : (m + 1) * 8, 0:HD
                    ]
                    srcv = pv.rearrange("p (h c) -> p h c", c=HD)
                    nc.vector.tensor_copy(dst, srcv)

            # RoPE helper: per 128-row tile the layout is [h0r, h0i, h1r,
            # h1i] (32 rows each); rot = t*cosf + swap32(t)*sinf (sinf
            # carries the sign)
            def rope(buf_t):
                tr = work.tile([P, S], BF, tag="trot", name="trot")
                for b4 in range(4):
                    sblk = (b4 ^ 1) * 32
                    dblk = b4 * 32
                    nc.sync.dma_start(
                        tr[dblk : dblk + 32, :], buf_t[sblk : sblk + 32, :]
                    )
                nc.vector.tensor_tensor(tr[:], tr[:], sinf[:], op=MUL)
                nc.vector.tensor_tensor(buf_t[:], buf_t[:], cosf[:], op=MUL)
                nc.vector.tensor_tensor(buf_t[:], buf_t[:], tr[:], op=ADD)

            # ---- QT/KT projections per tile; rope immediately per tile ----
            for t in range(NT):
                cs = slice(t * P, (t + 1) * P)
                for m in range(2):
                    sl = slice(m * 512, (m + 1) * 512)
                    pq = ps.tile([P, 512], F32, tag="ps", name="pq")
                    for j in range(NT):
                        nc.tensor.matmul(
                            pq[:], wqt[j][:, cs], xt[j][:, sl],
                            start=(j == 0), stop=(j == NT - 1),
                        )
                    nc.vector.tensor_copy(qt[t][:, sl], pq[:])
                    pk = ps.tile([P, 512], F32, tag="ps", name="pk")
                    for j in range(NT):
                        nc.tensor.matmul(
                            pk[:], wkt[j][:, cs], xt[j][:, sl],
                            start=(j == 0), stop=(j == NT - 1),
                        )
                    nc.vector.tensor_copy(kt[t][:, sl], pk[:])
                rope(qt[t])
                rope(kt[t])

            # ---- attention per head-pair; scores transposed ST[k, q] ----
            # the two heads (rows 0:64 and 64:128 of tile t) run their ST
            # matmuls concurrently in the PE (row groups 0/1 vs 2/3).
            scale = 1.0 / 8.0
            for t in range(NT):
                for hh in (0, 1):
                    h = 2 * t + hh
                    base = hh * HD
                    otp = ot_ps.tile([HC, S], F32, tag="ot", name="otp")
                    pts = []
                    for i in range(NT):
                        off = i * P
                        w = S - off
                        pt = ptp.tile([P, w], BF, tag=f"pt{i}", name=f"pt{i}")
                        pts.append(pt)
                        stp = ps.tile([P, 1024], F32, tag="ps", name="stp")
                        pieces = [(off, min(512, w))]
                        if w > 512:
                            pieces.append((off + 512, w - 512))
                        for (o, wd) in pieces:
                            nc.tensor.matmul(
                                stp[:, o - off : o - off + wd],
                                kt[t][base : base + HD, off : off + P],
                                qt[t][base : base + HD, o : o + wd],
                                start=True, stop=True,
                            )
                        nc.scalar.activation(
                            pt[:, :], stp[:, :w], EXP, scale=scale
                        )
                        nc.vector.tensor_tensor(
                            pt[:, 0:P], pt[:, 0:P], maskt[:], op=MUL
                        )
                    for jb in range(2):
                        lo = jb * 512
                        last_i = min(NT - 1, 4 * jb + 3)
                        for i in range(last_i + 1):
                            off = i * P
                            o = max(lo, off)
                            wd = lo + 512 - o
                            nc.tensor.matmul(
                                otp[:, o : o + wd],
                                vs[i][:, h * HC : (h + 1) * HC],
                                pts[i][:, o - off : o - off + wd],
                                start=(i == 0), stop=(i == last_i),
                            )
                    # normalization: rec = 1/den, bcast over 64 partitions,
                    # fused (copy + multiply) PSUM -> SBUF
                    den = work.tile([1, S], F32, tag="den", name="den")
                    nc.vector.tensor_copy(den[:], otp[HD : HD + 1, :])
                    rec = work.tile([1, S], F32, tag="rec", name="rec")
                    nc.vector.reciprocal_approx_fast(rec[:], den[:])
                    bc = work.tile([HD, S], F32, tag="bc", name="bc")
                    nc.gpsimd.partition_broadcast(bc[:], rec[:])
                    nc.vector.tensor_tensor(
                        ot[t][base : base + HD, :], otp[0:HD, :], bc[:], op=MUL
                    )

            # ---- output projection: final[s, :] = sum_i ot[i][:, s]^T wo[i]
            for st in range(NT):
                cs = slice(st * P, (st + 1) * P)
                fp = ps.tile([P, 1024], F32, tag="ps", name="fp")
                for i in range(NT):
                    for m in range(2):
                        sl = slice(m * 512, (m + 1) * 512)
                        nc.tensor.matmul(
                            fp[:, sl], ot[i][:, cs], wot[i][:, sl],
                            start=(i == 0), stop=(i == NT - 1),
                        )
                osb = work.tile([P, 1024], F32, tag="osb", name="osb")
                nc.vector.tensor_copy(osb[:], fp[:])
                nc.sync.dma_start(out_d[cs, :], osb[:])


    nc.compile()
    _NC_CACHE["nc"] = nc
    return nc


def _host_prep(x, wq, wk, wv, wo, freqs_cos, freqs_sin):
    import ml_dtypes

    bf16 = ml_dtypes.bfloat16

    # de-interleave RoPE pairs: permuted col c of head h maps to original
    # column h*64 + (2r if r<32 else 2(r-32)+1)
    r = np.arange(HD)
    src_local = np.where(r < 32, 2 * r, 2 * (r - 32) + 1)
    perm = (np.arange(H)[:, None] * HD + src_local[None, :]).reshape(-1)

    wq_p = np.ascontiguousarray(wq[:, perm]).astype(bf16)
    wk_p = np.ascontiguousarray(wk[:, perm]).astype(bf16)
    wv_c = np.ascontiguousarray(wv).astype(bf16)
    wo_c = np.ascontiguousarray(wo).astype(bf16)

    cos_t = np.ascontiguousarray(freqs_cos.T).astype(np.float32)  # [32, S]
    sin_t = np.ascontiguousarray(freqs_sin.T).astype(np.float32)
    cosf = np.concatenate([cos_t, cos_t, cos_t, cos_t], 0).astype(bf16)  # [128,S]
    sinf = np.concatenate([-sin_t, sin_t, -sin_t, sin_t], 0).astype(bf16)

    kq = np.arange(P)
    mask = ((kq[:, None] // BS) <= (kq[None, :] // BS)).astype(bf16)  # [128,128]

    in_maps = []
    for b in range(NCORES):
        xt = np.ascontiguousarray(x[b].T).astype(bf16)  # [D, S]
        in_maps.append(
            {
                "xt": xt,
                "wq": wq_p,
                "wk": wk_p,
                "wv": wv_c,
                "wo": wo_c,
                "cosf": cosf,
                "sinf": sinf,
                "mask": mask,
            }
        )
    return in_maps


def kernel(x, wq, wk, wv, wo, freqs_cos, freqs_sin):
    global LAST_RESULT
    x = np.asarray(x, dtype=np.float32)
    wq = np.asarray(wq, dtype=np.float32)
    wk = np.asarray(wk, dtype=np.float32)
    wv = np.asarray(wv, dtype=np.float32)
    wo = np.asarray(wo, dtype=np.float32)
    freqs_cos = np.asarray(freqs_cos, dtype=np.float32)
    freqs_sin = np.asarray(freqs_sin, dtype=np.float32)

    trace = bool(os.environ.get("BASS_TRACE"))
    if trace:
        _install_axon_hooks()
        import concourse.bass_utils as bass_utils

        bass_utils.upload_artifacts = lambda tmpdir: tmpdir  # no-egress sandbox

    from concourse.bass_utils import run_bass_kernel_spmd

    nc = _build_nc()
    in_maps = _host_prep(x, wq, wk, wv, wo, freqs_cos, freqs_sin)
    res = run_bass_kernel_spmd(
        nc, in_maps, core_ids=list(range(NCORES)), trace=trace
    )
    LAST_RESULT = res
    out = np.stack([res.results[b]["out"] for b in range(NCORES)], 0)
    return out.astype(np.float32)


# revision 21
# speedup vs baseline: 1.8214x; 1.1244x over previous
"""Self-contained Trainium2 Bass kernel for batched multi-head attention
with interleaved RoPE and a block-causal mask (block size 8).

Shapes (hardcoded): x [8, 1024, 1024] f32, weights [1024, 1024] f32,
freqs_cos/sin [1024, 32] f32 -> out [8, 1024, 1024] f32.

Sharding: data-parallel over batch, one batch element per NeuronCore (8 cores).

Device algorithm (per core, matmuls in bf16):
  - host pre-transposes x -> XT [D, S] and de-interleaves the RoPE pairing by
    permuting wq/wk columns so each head's 64 dims are [32 real | 32 imag].
  - QT = Wq^T XT, KT = Wk^T XT  ([D, S] layouts, head-major rows)
  - RoPE in [d, s] layout: rot = t * cosf + swap32(t) * sinf, with the 32-row
    block swap done by SBUF->SBUF DMA and sign folded into the sinf table.
  - V = XT^T Wv in natural [S, D] layout, stored with a ones-column per head
    (V' [S, 65] per head) so the PV matmul also produces the softmax
    denominator as its row 64.
  - scores transposed: ST[k, q] = (KT_h k-slice)^T @ QT_h per head per
    128-wide k tile, staircase over allowed q only; the two heads of a
    partition tile run concurrently in the PE via row-group packing.
    exp on ACT with the 1/8 scale folded in; block-diagonal mask applied
    multiplicatively.
  - outT[h] = V'^T @ PT accumulated over k tiles in PSUM ([65, S]).
  - normalization per head: rec = reciprocal_approx(den row) (custom DVE op),
    partition-broadcast on GPSIMD, single fused TT multiply PSUM->SBUF.
  - final = outT^T @ Wo streamed back to HBM in f32.

Matmul chains are emitted with independent accumulations interleaved so the
PE's reorder window can hide LDWEIGHTS behind the previous matmul's stream.
"""

import os
import sys
import types

import numpy as np

B, S, D, H, HD, BS = 8, 1024, 1024, 16, 64, 8
P = 128
NT = D // P  # 8 partition tiles
NCORES = 8

LAST_RESULT = None  # BassKernelResults of the most recent run (for test harness)


def _install_axon_hooks():
    """Provide antenv.axon_hooks (NTFF profiling hook) when the image lacks it."""
    if "antenv.axon_hooks" in sys.modules:
        return
    try:
        import antenv
        from trn_agent_boot.trn_boot import _ntff_profile_via_ctypes

        mod = types.ModuleType("antenv.axon_hooks")
        hook = _ntff_profile_via_ctypes("/opt/axon/libaxon_pjrt.so")
        mod.get_axon_ntff_profile_hook = lambda: hook
        mod.set_axon_ntff_profile_hook = lambda h: None
        sys.modules["antenv.axon_hooks"] = mod
        antenv.axon_hooks = mod
    except Exception:
        mod = types.ModuleType("antenv.axon_hooks")
        mod.get_axon_ntff_profile_hook = lambda: None
        mod.set_axon_ntff_profile_hook = lambda h: None
        sys.modules["antenv.axon_hooks"] = mod


_NC_CACHE = {}


def _build_nc():
    """Build and compile the Bass graph (one SPMD program for all 8 cores)."""
    if "nc" in _NC_CACHE:
        return _NC_CACHE["nc"]

    import concourse.mybir as mybir
    import concourse.tile as tile
    from concourse import bacc

    BF = mybir.dt.bfloat16
    F32 = mybir.dt.float32
    MUL = mybir.AluOpType.mult
    ADD = mybir.AluOpType.add
    EXP = mybir.ActivationFunctionType.Exp

    nc = bacc.Bacc("TRN2", target_bir_lowering=False, debug=False)

    xt_d = nc.dram_tensor("xt", [D, S], BF, kind="ExternalInput")
    wq_d = nc.dram_tensor("wq", [D, D], BF, kind="ExternalInput")
    wk_d = nc.dram_tensor("wk", [D, D], BF, kind="ExternalInput")
    wv_d = nc.dram_tensor("wv", [D, D], BF, kind="ExternalInput")
    wo_d = nc.dram_tensor("wo", [D, D], BF, kind="ExternalInput")
    cos_d = nc.dram_tensor("cosf", [P, S], BF, kind="ExternalInput")
    sin_d = nc.dram_tensor("sinf", [P, S], BF, kind="ExternalInput")
    mask_d = nc.dram_tensor("mask", [P, P], BF, kind="ExternalInput")
    out_d = nc.dram_tensor("out", [S, D], F32, kind="ExternalOutput")

    HC = HD + 1  # 65: V columns per head incl. the ones column

    with tile.TileContext(nc) as tc:
        with (
            tc.tile_pool(name="big", bufs=1) as big,
            tc.tile_pool(name="ps", bufs=2, space="PSUM") as ps,
            tc.tile_pool(name="ot_ps", bufs=2, space="PSUM") as ot_ps,
            tc.tile_pool(name="work", bufs=2) as work,
            tc.tile_pool(name="ptp", bufs=2) as ptp,
        ):
            xt = [big.tile([P, S], BF, tag=f"xt{j}", name=f"xt{j}") for j in range(NT)]
            wqt = [big.tile([P, D], BF, tag=f"wq{j}", name=f"wq{j}") for j in range(NT)]
            wkt = [big.tile([P, D], BF, tag=f"wk{j}", name=f"wk{j}") for j in range(NT)]
            wvt = [big.tile([P, D], BF, tag=f"wv{j}", name=f"wv{j}") for j in range(NT)]
            wot = [big.tile([P, D], BF, tag=f"wo{j}", name=f"wo{j}") for j in range(NT)]
            qt = [big.tile([P, S], BF, tag=f"qt{t}", name=f"qt{t}") for t in range(NT)]
            kt = [big.tile([P, S], BF, tag=f"kt{t}", name=f"kt{t}") for t in range(NT)]
            vs = [big.tile([P, H * HC], BF, tag=f"vs{t}", name=f"vs{t}") for t in range(NT)]
            ot = [big.tile([P, S], BF, tag=f"ot{t}", name=f"ot{t}") for t in range(NT)]
            cosf = big.tile([P, S], BF, tag="cosf", name="cosf")
            sinf = big.tile([P, S], BF, tag="sinf", name="sinf")
            maskt = big.tile([P, P], BF, tag="mask", name="mask")

            # load order matters for the compute ramp: xt+wv feed the V
            # projection (needed before any attention), wq/wk next, wo last
            for j in range(NT):
                rs = slice(j * P, (j + 1) * P)
                nc.sync.dma_start(xt[j][:], xt_d[rs, :])
                nc.sync.dma_start(wvt[j][:], wv_d[rs, :])
                nc.sync.dma_start(wqt[j][:], wq_d[rs, :])
                nc.sync.dma_start(wkt[j][:], wk_d[rs, :])
            nc.sync.dma_start(cosf[:], cos_d[:])
            nc.sync.dma_start(sinf[:], sin_d[:])
            nc.sync.dma_start(maskt[:], mask_d[:])
            for j in range(NT):
                rs = slice(j * P, (j + 1) * P)
                nc.sync.dma_start(wot[j][:], wo_d[rs, :])

            for t in range(NT):
                nc.vector.memset(
                    vs[t].rearrange("p (h c) -> p h c", c=HC)[:, :, HD : HD + 1], 1.0
                )

            # ---- V projection first (all of V gates the first head's PV) --
            for t in range(NT):
                cs = slice(t * P, (t + 1) * P)
                for m in range(2):
                    sl = slice(m * 512, (m + 1) * 512)
                    pv = ps.tile([P, 512], F32, tag="ps1", name="pv")
                    for j in range(NT):
                        nc.tensor.matmul(
                            pv[:], xt[j][:, cs], wvt[j][:, sl],
                            start=(j == 0), stop=(j == NT - 1),
                        )
                    dst = vs[t].rearrange("p (h c) -> p h c", c=HC)[
                        :, m * 8 : (m + 1) * 8, 0:HD
                    ]
                    srcv = pv.rearrange("p (h c) -> p h c", c=HD)
                    nc.vector.tensor_copy(dst, srcv)

            # RoPE helper: per 128-row tile the layout is [h0r, h0i, h1r,
            # h1i] (32 rows each); rot = t*cosf + swap32(t)*sinf (sinf
            # carries the sign)
            def rope(buf_t):
                tr = work.tile([P, S], BF, tag="trot", name="trot")
                for b4 in range(4):
                    sblk = (b4 ^ 1) * 32
                    dblk = b4 * 32
                    nc.sync.dma_start(
                        tr[dblk : dblk + 32, :], buf_t[sblk : sblk + 32, :]
                    )
                nc.vector.tensor_tensor(tr[:], tr[:], sinf[:], op=MUL)
                nc.vector.tensor_tensor(buf_t[:], buf_t[:], cosf[:], op=MUL)
                nc.vector.tensor_tensor(buf_t[:], buf_t[:], tr[:], op=ADD)

            # ---- QT/KT projections per tile; rope immediately per tile ----
            for t in range(NT):
                cs = slice(t * P, (t + 1) * P)
                for m in range(2):
                    sl = slice(m * 512, (m + 1) * 512)
                    pq = ps.tile([P, 512], F32, tag="ps1", name="pq")
                    for j in range(NT):
                        nc.tensor.matmul(
                            pq[:], wqt[j][:, cs], xt[j][:, sl],
                            start=(j == 0), stop=(j == NT - 1),
                        )
                    nc.vector.tensor_copy(qt[t][:, sl], pq[:])
                    pk = ps.tile([P, 512], F32, tag="ps1", name="pk")
                    for j in range(NT):
                        nc.tensor.matmul(
                            pk[:], wkt[j][:, cs], xt[j][:, sl],
                            start=(j == 0), stop=(j == NT - 1),
                        )
                    nc.vector.tensor_copy(kt[t][:, sl], pk[:])
                rope(qt[t])
                rope(kt[t])

            # ---- attention per head-pair; scores transposed ST[k, q] ----
            # the two heads (rows 0:64 and 64:128 of tile t) run their ST
            # matmuls concurrently in the PE (row groups 0/1 vs 2/3).
            scale = 1.0 / 8.0
            for t in range(NT):
                for hh in (0, 1):
                    h = 2 * t + hh
                    base = hh * HD
                    otp = {
                        0: ot_ps.tile([HC, 512], F32, tag="ot", name="otp0"),
                        1: ot_ps.tile([HC, 512], F32, tag="ot", name="otp1"),
                    }

                    def norm_bank(jb):
                        # rec = 1/den, bcast over 64 partitions, fused
                        # (copy + multiply) PSUM -> SBUF
                        sl = slice(jb * 512, (jb + 1) * 512)
                        den = work.tile([1, 512], F32, tag="den", name="den")
                        nc.vector.tensor_copy(den[:], otp[jb][HD : HD + 1, :])
                        rec = work.tile([1, 512], F32, tag="rec", name="rec")
                        nc.vector.reciprocal_approx_fast(rec[:], den[:])
                        bc = work.tile([HD, 512], F32, tag="bc", name="bc")
                        nc.gpsimd.partition_broadcast(bc[:], rec[:])
                        nc.vector.tensor_tensor(
                            ot[t][base : base + HD, sl], otp[jb][0:HD, :], bc[:],
                            op=MUL,
                        )

                    for i in range(NT):
                        off = i * P
                        w = S - off
                        pt = ptp.tile([P, w], BF, tag=f"pt{i}", name=f"pt{i}")
                        if w > 512:
                            stp = ps.tile([P, 1024], F32, tag="ps", name="stp")
                        else:
                            stp = ps.tile([P, 512], F32, tag="ps1", name="stp1")
                        pieces = [(off, min(512, w))]
                        if w > 512:
                            pieces.append((off + 512, w - 512))
                        for (o, wd) in pieces:
                            nc.tensor.matmul(
                                stp[:, o - off : o - off + wd],
                                kt[t][base : base + HD, off : off + P],
                                qt[t][base : base + HD, o : o + wd],
                                start=True, stop=True,
                            )
                        nc.scalar.activation(
                            pt[:, :], stp[:, :w], EXP, scale=scale
                        )
                        nc.vector.tensor_tensor(
                            pt[:, 0:P], pt[:, 0:P], maskt[:], op=MUL
                        )
                        for jb in range(2):
                            lo = jb * 512
                            if i > 4 * jb + 3:
                                continue
                            o = max(lo, off)
                            wd = lo + 512 - o
                            nc.tensor.matmul(
                                otp[jb][:, o - lo : o - lo + wd],
                                vs[i][:, h * HC : (h + 1) * HC],
                                pt[:, o - off : o - off + wd],
                                start=(i == 0), stop=(i == min(NT - 1, 4 * jb + 3)),
                            )
                        if i == 3:
                            norm_bank(0)
                    norm_bank(1)

            # ---- output projection: final[s, :] = sum_i ot[i][:, s]^T wo[i]
            for st in range(NT):
                cs = slice(st * P, (st + 1) * P)
                for m in range(2):
                    sl = slice(m * 512, (m + 1) * 512)
                    fp = ps.tile([P, 512], F32, tag="ps1", name="fp")
                    for i in range(NT):
                        nc.tensor.matmul(
                            fp[:], ot[i][:, cs], wot[i][:, sl],
                            start=(i == 0), stop=(i == NT - 1),
                        )
                    osb = work.tile([P, 512], F32, tag="osb", name="osb")
                    nc.vector.tensor_copy(osb[:], fp[:])
                    nc.sync.dma_start(out_d[cs, sl], osb[:])


    nc.compile()
    _NC_CACHE["nc"] = nc
    return nc


def _host_prep(x, wq, wk, wv, wo, freqs_cos, freqs_sin):
    import ml_dtypes

    bf16 = ml_dtypes.bfloat16

    # de-interleave RoPE pairs: permuted col c of head h maps to original
    # column h*64 + (2r if r<32 else 2(r-32)+1)
    r = np.arange(HD)
    src_local = np.where(r < 32, 2 * r, 2 * (r - 32) + 1)
    perm = (np.arange(H)[:, None] * HD + src_local[None, :]).reshape(-1)

    wq_p = np.ascontiguousarray(wq[:, perm]).astype(bf16)
    wk_p = np.ascontiguousarray(wk[:, perm]).astype(bf16)
    wv_c = np.ascontiguousarray(wv).astype(bf16)
    wo_c = np.ascontiguousarray(wo).astype(bf16)

    cos_t = np.ascontiguousarray(freqs_cos.T).astype(np.float32)  # [32, S]
    sin_t = np.ascontiguousarray(freqs_sin.T).astype(np.float32)
    cosf = np.concatenate([cos_t, cos_t, cos_t, cos_t], 0).astype(bf16)  # [128,S]
    sinf = np.concatenate([-sin_t, sin_t, -sin_t, sin_t], 0).astype(bf16)

    kq = np.arange(P)
    mask = ((kq[:, None] // BS) <= (kq[None, :] // BS)).astype(bf16)  # [128,128]

    in_maps = []
    for b in range(NCORES):
        xt = np.ascontiguousarray(x[b].T).astype(bf16)  # [D, S]
        in_maps.append(
            {
                "xt": xt,
                "wq": wq_p,
                "wk": wk_p,
                "wv": wv_c,
                "wo": wo_c,
                "cosf": cosf,
                "sinf": sinf,
                "mask": mask,
            }
        )
    return in_maps


def kernel(x, wq, wk, wv, wo, freqs_cos, freqs_sin):
    global LAST_RESULT
    x = np.asarray(x, dtype=np.float32)
    wq = np.asarray(wq, dtype=np.float32)
    wk = np.asarray(wk, dtype=np.float32)
    wv = np.asarray(wv, dtype=np.float32)
    wo = np.asarray(wo, dtype=np.float32)
    freqs_cos = np.asarray(freqs_cos, dtype=np.float32)
    freqs_sin = np.asarray(freqs_sin, dtype=np.float32)

    trace = bool(os.environ.get("BASS_TRACE"))
    if trace:
        _install_axon_hooks()
        import concourse.bass_utils as bass_utils

        bass_utils.upload_artifacts = lambda tmpdir: tmpdir  # no-egress sandbox

    from concourse.bass_utils import run_bass_kernel_spmd

    nc = _build_nc()
    in_maps = _host_prep(x, wq, wk, wv, wo, freqs_cos, freqs_sin)
    res = run_bass_kernel_spmd(
        nc, in_maps, core_ids=list(range(NCORES)), trace=trace
    )
    LAST_RESULT = res
    out = np.stack([res.results[b]["out"] for b in range(NCORES)], 0)
    return out.astype(np.float32)
